# revision 67
# baseline (speedup 1.0000x reference)
"""Distributed Trainium2 Bass kernel for AdaptedAttention (LLaMA-Adapter style).

Sharding: pure data-parallel over the B*S = 8192 token axis (1024 tokens per
core across 8 NeuronCores).  The adapter attention only attends to the L=64
adapter slots, so there is no cross-token dependency; each core produces its
own slice of the output.

Weight folding (host, numpy): the adapter K and the product V@Wo depend only
on weight-type inputs (adaption_prompt, Wk, Wv, Wo, adaption_gate), so they
are folded on the host like the other weight transforms (gate into Wv, scale
into Wq, RoPE tables).  Folding V@Wo per head gives a [H*L=2048, HID] output
projection, halving the O-proj FLOPs (the contraction drops 4096 -> 2048) and
removing the per-head probs@V matmuls entirely.

Device pipeline per core (all big matmuls fp8e4m3 DoubleRow, K=256/instr):
  - per head pair a (heads 2a, 2a+1): qT = WqT^T @ xT (PSUM), RoPE arms
    qa=q*cos, qb=q*sin' as fp8 (rotate-half eliminated: scores contract over
    the head dim, so scores = KT^T qa + KTrowswap^T qb, one DoubleRow mm with
    [KT|KTs] as the two K-groups); both heads of a pair land in one
    [128,512] PSUM tile -> one Exp activation.
  - softmax sums batched: an indicator-matrix matmul accumulates per-head
    column sums of exp into a shared [16,512] PSUM tile (value 1/64 folds the
    fp8 probs scale); one reciprocal per 16-head group replaces 64 tiny ones.
  - reciprocal rows are partition-broadcast with a second indicator matmul
    (PE, ~0.2us) and multiplied into probs (fp8, scale 64).
  - O-proj: outT = (VWo^T probsT) * OSCALE + baseT via fp8 DoubleRow matmuls
    over the (head,slot) contraction.
All stages are software-pipelined (scores lag 1 pair, sums lag 2, probs of
group 0 interleave with the Qproj of group 1) so TensorE never waits on
Scalar/Vector.
"""

import numpy as np
import ml_dtypes

B, S, HID = 4, 2048, 4096
H, D, L = 32, 128, 64
NCORES = 8
T = B * S
TC = T // NCORES          # tokens per core (1024)
KC = HID // 128           # 32 contraction chunks over hidden dim
HL = H * L                # folded O-proj contraction (2048)
MB = TC // 512            # 512-token m-chunks (2)
ROPE_THETA = 10000.0

S_X = 16.0                # fp8 scale on xT
S_Q = 8192.0              # fp8 scale on WqT (1/sqrt(D) already folded)
S_P = 16.0                # fp8 scale on qa/qb (rope products)
S_K = 16.0                # fp8 scale on adapter KT
S_PR = 64.0               # fp8 scale on probs (folded via 1/64 indicator)
S_VW = 1024.0             # fp8 scale on VWo
QSCALE = S_P / (S_X * S_Q)   # folded into the cos/sin tables on host
ESCALE = 1.0 / (S_P * S_K)   # descale via the exp activation's scale arg
OSCALE = 1.0 / (S_PR * S_VW)
SWI = True                   # DoubleRowSwInterleave weight layout (fast LDW)

_cache = {}


def _build(tc_tokens=TC):
    """Builds the SPMD Bass graph (identical on all 8 cores)."""
    import concourse.tile as tile
    from concourse import bacc, mybir
    from contextlib import ExitStack

    bf16 = mybir.dt.bfloat16
    fp8 = mybir.dt.float8e4
    f32 = mybir.dt.float32
    MUL = mybir.AluOpType.mult
    ADD = mybir.AluOpType.add
    EXP = mybir.ActivationFunctionType.Exp
    DR = (mybir.MatmulPerfMode.DoubleRowSwInterleave if SWI
          else mybir.MatmulPerfMode.DoubleRow)

    assert tc_tokens == TC and MB == 2

    nc = bacc.Bacc(
        "TRN2",
        target_bir_lowering=False,
        debug=False,
        enable_asserts=False,
    )

    # Host-pretiled layouts (every DMA a large contiguous burst):
    #   xT    [128, KC*tc]   : DoubleRow rhs tiling (see tile_dr_rhs)
    #   wqT   [H*128, KC*128]: [128h+p, 256k2+128i+c] = (Wq.T*s)[256k2+128i+p, 128h+c]
    #   ktpT  [128, H*2*L]   : [p, 128h+64i+l] = KT_h[p,l] (i=0) / KT_h[(p+64)%128,l] (i=1)
    #   vwoT  [KC*128, HL]   : [128n+p, 256b+128i+c] = (VWo*s)[256b+128i+p, 128n+c]
    xT = nc.dram_tensor("xT", [128, KC * TC], fp8, kind="ExternalInput").ap()
    cosT = nc.dram_tensor("cosT", [D, TC], bf16, kind="ExternalInput").ap()
    sinT = nc.dram_tensor("sinT", [D, TC], bf16, kind="ExternalInput").ap()
    wqT = nc.dram_tensor("wqT", [H * 128, KC * 128], fp8, kind="ExternalInput").ap()
    ktpT = nc.dram_tensor("ktpT", [128, H * 2 * L], fp8, kind="ExternalInput").ap()
    vwoT = nc.dram_tensor("vwoT", [KC * 128, HL], fp8, kind="ExternalInput").ap()
    baseT = nc.dram_tensor("baseT", [HID, TC], bf16, kind="ExternalInput").ap()
    u2T = nc.dram_tensor("u2T", [128, 64], bf16, kind="ExternalInput").ap()
    z2T = nc.dram_tensor("z2T", [32, 33 * 64], bf16, kind="ExternalInput").ap()
    outT = nc.dram_tensor("outT", [HID, TC], f32, kind="ExternalOutput").ap()

    with tile.TileContext(nc) as tc, ExitStack() as ctx:
        const_pool = ctx.enter_context(tc.tile_pool(name="const", bufs=1))
        persist = ctx.enter_context(tc.tile_pool(name="persist", bufs=1))

        # ---- persistent SBUF residents ----
        xT_sb = persist.tile([128, KC * TC], fp8)
        cos_sb = persist.tile([128, TC], bf16)
        sin_sb = persist.tile([128, TC], bf16)
        ktp_sb = persist.tile([128, H * 2 * L], fp8)
        probsT_sb = persist.tile([128, 16 * 1024], fp8)   # [p, 1024a+512m+t]
        vwo_sb = persist.tile([128, KC * HL], fp8)        # [p, 2048ni+col]
        U2 = const_pool.tile([128, 64], bf16)
        Z2 = const_pool.tile([32, 33 * 64], bf16)

        xT_r = xT_sb.rearrange("p (k q i m) -> p k q i m", k=KC // 2, q=MB, i=2)
        xTd_r = xT.rearrange("p (k q i m) -> p k q i m", k=KC // 2, q=MB, i=2)
        ktp_r = ktp_sb.rearrange("p (h i l) -> p h i l", h=H, i=2)

        def x_load(eng, m, k2a, k2b):
            # strided load of one token-half's k2 range (1024-byte runs)
            eng.dma_start(xT_r[:, k2a:k2b, m], xTd_r[:, k2a:k2b, m])

        def late_loads():
            # split across both HWDGE queues (sync + Activation) so
            # descriptor issue and transfers parallelize
            nc.scalar.dma_start(cos_sb[:], cosT[:])
            nc.scalar.dma_start(sin_sb[:], sinT[:])
            nc.scalar.dma_start(ktp_sb[:], ktpT[:])
            nc.scalar.dma_start(U2[:], u2T[:])
            nc.scalar.dma_start(Z2[:], z2T[:])
            x_load(nc.sync, 1, 0, 8)
            x_load(nc.sync, 1, 8, 16)

        # ============ Phase B: Qproj + RoPE + scores + softmax ============
        with tc.tile_pool(name="wq", bufs=2) as wqp, \
             tc.tile_pool(name="rope", bufs=10) as rp, \
             tc.tile_pool(name="esb", bufs=34) as esbp, \
             tc.tile_pool(name="recs", bufs=4) as recp, \
             tc.tile_pool(name="fin", bufs=4) as fin, \
             tc.tile_pool(name="sups", bufs=1, space="PSUM") as sup, \
             tc.tile_pool(name="bcps", bufs=2, space="PSUM") as bcp:

            qab_st, esb_st, sums_st, rec_st, wq_st = {}, {}, {}, {}, {}

            def wq_load(j, split=False):
                wq_sb = wqp.tile([128, KC * 128], fp8, tag="wq")
                if split:   # first head: let LDW k2=0 start after half the dma
                    nc.sync.dma_start(wq_sb[:, 0:2048],
                                      wqT[128 * j:128 * (j + 1), 0:2048])
                    nc.sync.dma_start(wq_sb[:, 2048:4096],
                                      wqT[128 * j:128 * (j + 1), 2048:4096])
                else:
                    nc.sync.dma_start(wq_sb[:], wqT[128 * j:128 * (j + 1), :])
                wq_st[j] = wq_sb

            def qproj(a):
                for j in (2 * a, 2 * a + 1):
                    wq_sb = wq_st.pop(j)
                    wq_r = wq_sb.rearrange("p (k i c) -> p k i c",
                                           k=KC // 2, i=2)
                    for m in range(MB):
                        qp = qpsp.tile([128, 512], f32, tag="qp",
                                       name=f"qp{j}_{m}")
                        for k2 in range(KC // 2):
                            lhsT = (wq_sb[:, 256 * k2:256 * (k2 + 1)]
                                    if SWI else wq_r[:, k2])
                            nc.tensor.matmul(
                                qp[:], lhsT, xT_r[:, k2, m],
                                start=(k2 == 0), stop=(k2 == KC // 2 - 1),
                                perf_mode=DR,
                            )
                        ms = slice(512 * m, 512 * (m + 1))
                        qab = rp.tile([128, 1024], fp8, tag="qab",
                                      name=f"qab{j}_{m}")
                        nc.vector.tensor_tensor(qab[:, 0:512], qp[:],
                                                cos_sb[:, ms], MUL)
                        nc.vector.tensor_tensor(qab[:, 512:1024], qp[:],
                                                sin_sb[:, ms], MUL)
                        qab_st[(j, m)] = qab

            def scores(a):
                # DoubleRow can't col-tile (XBUS budget), so the two RoPE
                # arms accumulate as two plain fp8 matmuls per head; the two
                # heads of a pair land in col-groups 0-63 / 64-127.
                for m in range(MB):
                    psc = scp.tile([128, 512], f32, tag="sc", name=f"sc{a}_{m}")
                    for i, j in enumerate((2 * a, 2 * a + 1)):
                        qab = qab_st.pop((j, m))
                        for arm in range(2):
                            nc.tensor.matmul(
                                psc[64 * i:64 * (i + 1), :],
                                ktp_r[:, j, arm, :],
                                qab[:, 512 * arm:512 * (arm + 1)],
                                start=(arm == 0), stop=(arm == 1),
                            )
                    esb = esbp.tile([128, 512], bf16, tag="esb",
                                    name=f"esb{a}_{m}")
                    nc.scalar.activation(esb[:], psc[:], EXP, scale=ESCALE)
                    esb_st[(a, m)] = esb

            def sums(a):
                # Both m-chunks of a group share one [32,512] PSUM bank
                # (head 2al+i of chunk m at row 2al+16m+i) -> one reciprocal
                # per 16-head group.
                g, al = a // 8, a % 8
                if al == 0:
                    sums_st[g] = sup.tile([32, 512], f32, tag="sums",
                                          name=f"su{g}")
                for m in range(MB):
                    r0 = 2 * al + 16 * m
                    nc.tensor.matmul(
                        sums_st[g][:], U2[:, 31 - r0:63 - r0],
                        esb_st[(a, m)][:],
                        start=(al == 0 and m == 0), stop=(al == 7 and m == 1),
                    )

            def rec(g):
                rc = recp.tile([32, 512], bf16, tag="rec", name=f"rec{g}")
                with nc.allow_low_precision(reason="bf16 softmax weights"):
                    nc.vector.reciprocal(rc[:], sums_st[g][:])
                rec_st[g] = rc

            def bc_probs(a):
                g, al = a // 8, a % 8
                for m in range(MB):
                    r0 = 2 * al + 16 * m
                    pb = bcp.tile([128, 512], f32, tag="bcp", name=f"bc{a}_{m}")
                    # two M=64 matmuls in different col-groups run
                    # concurrently (same trick as the scores pair)
                    for i in range(2):
                        nc.tensor.matmul(
                            pb[64 * i:64 * (i + 1), :],
                            Z2[:, 64 * (r0 + i):64 * (r0 + i) + 64],
                            rec_st[g][:], start=True, stop=True)
                    ps = slice(1024 * a + 512 * m, 1024 * a + 512 * (m + 1))
                    nc.vector.tensor_tensor(probsT_sb[:, ps],
                                            esb_st.pop((a, m))[:], pb[:], MUL)

            # Qproj/scores PSUM pools nest so their 4 banks free up for the
            # O-proj tiles that overlap the group-1 softmax tail.
            with tc.tile_pool(name="qps", bufs=3, space="PSUM") as qpsp, \
                 tc.tile_pool(name="scps", bufs=2, space="PSUM") as scp:
                # head-0 loads interleaved so the first matmul waits only on
                # the first wq half + first xT chunk
                wq_sb0 = wqp.tile([128, KC * 128], fp8, tag="wq")
                wq_st[0] = wq_sb0
                nc.sync.dma_start(wq_sb0[:, 0:2048], wqT[0:128, 0:2048])
                x_load(nc.sync, 0, 0, 2)
                nc.sync.dma_start(wq_sb0[:, 2048:4096], wqT[0:128, 2048:4096])
                x_load(nc.sync, 0, 2, 5)
                x_load(nc.scalar, 0, 5, 10)
                x_load(nc.scalar, 0, 10, 16)
                wq_load(1)
                late_loads()
                for a in range(16):
                    qproj(a)
                    if a < 15:      # prefetch next pair's weights
                        wq_load(2 * a + 2)
                        wq_load(2 * a + 3)
                    # stream the resident O-proj weights in during phase B
                    for ni in (2 * a, 2 * a + 1):
                        nc.scalar.dma_start(
                            vwo_sb[:, HL * ni:HL * (ni + 1)],
                            vwoT[128 * ni:128 * (ni + 1), :])
                    if a >= 1:
                        scores(a - 1)
                    if a == 10:
                        rec(0)  # before sums(8) so group-0 PSUM slots free
                    if a >= 2:
                        sums(a - 2)
                    if a >= 12:
                        bc_probs(2 * (a - 12))
                        bc_probs(2 * (a - 12) + 1)
                scores(15)
                sums(14)
                sums(15)

            # ============ Phase C: fp8 DoubleRow O-proj + base add ========
            # ni 0/1 accumulate their group-0 chunks (b 0-3) while the
            # group-1 reciprocal/broadcast/probs tail is still running.
            with tc.tile_pool(name="ops", bufs=4, space="PSUM") as opp:
                pr_r = probsT_sb.rearrange("p (b i m t) -> p b i m t",
                                           b=8, i=2, m=MB)
                vwo_r = vwo_sb.rearrange("p (n b i c) -> p n b i c",
                                         n=KC, b=8, i=2)

                def omm(op, ni, m, b, start, stop):
                    lhsT = (vwo_sb[:, HL * ni + 256 * b:HL * ni + 256 * (b + 1)]
                            if SWI else vwo_r[:, ni, b])
                    nc.tensor.matmul(op[:], lhsT, pr_r[:, b, :, m, :],
                                     start=start, stop=stop, perf_mode=DR)

                def ofin(ni, osb, op, m, bt):
                    ts = slice(512 * m, 512 * (m + 1))
                    nc.vector.scalar_tensor_tensor(
                        osb[:, ts], op[:], OSCALE, bt[:, ts], MUL, ADD)

                op_st, bt_st = {}, {}
                for ni in (0, 1):
                    bt = fin.tile([128, TC], bf16, tag="bt", name=f"bt{ni}")
                    nc.sync.dma_start(bt[:], baseT[128 * ni:128 * (ni + 1), :])
                    bt_st[ni] = bt
                    for m in range(MB):
                        op = opp.tile([128, 512], f32, tag="op",
                                      name=f"op{ni}_{m}")
                        op_st[(ni, m)] = op
                        for b in range(4):
                            omm(op, ni, m, b, b == 0, False)
                rec(1)
                for a in range(8, 16):
                    bc_probs(a)
                for ni in (0, 1):
                    osb = fin.tile([128, TC], f32, tag="osb")
                    bt = bt_st.pop(ni)
                    for m in range(MB):
                        op = op_st.pop((ni, m))
                        for b in range(4, 8):
                            omm(op, ni, m, b, False, b == 7)
                        ofin(ni, osb, op, m, bt)
                    nc.scalar.dma_start(outT[128 * ni:128 * (ni + 1), :],
                                        osb[:])

                for ni in range(2, KC):
                    bt = fin.tile([128, TC], bf16, tag="bt")
                    nc.sync.dma_start(bt[:], baseT[128 * ni:128 * (ni + 1), :])
                    osb = fin.tile([128, TC], f32, tag="osb")
                    for m in range(MB):
                        op = opp.tile([128, 512], f32, tag="op",
                                      name=f"op{ni}_{m}")
                        for b in range(8):
                            omm(op, ni, m, b, b == 0, b == 7)
                        ofin(ni, osb, op, m, bt)
                        if ni == KC - 1:
                            # last tile: write per-m so the final (smaller)
                            # DMA starts before the m=1 matmuls finish
                            ts = slice(512 * m, 512 * (m + 1))
                            nc.scalar.dma_start(
                                outT[128 * ni:128 * (ni + 1), ts],
                                osb[:, ts])
                    if ni < KC - 1:
                        nc.scalar.dma_start(
                            outT[128 * ni:128 * (ni + 1), :], osb[:])

    nc.compile()
    return nc


def _host_prep(hidden_states, base_output, Wq, Wk, Wv, Wo, adaption_prompt,
               adaption_gate, position_ids, tc_tokens=TC, ncores=NCORES):
    bf16 = ml_dtypes.bfloat16
    fp8 = ml_dtypes.float8_e4m3
    f32 = np.float32

    def to_fp8(a):
        return np.clip(a, -240.0, 240.0).astype(fp8)

    x = np.ascontiguousarray(np.asarray(hidden_states, f32).reshape(T, HID))
    base = np.asarray(base_output, f32).reshape(T, HID)
    pos = np.asarray(position_ids).reshape(T).astype(np.int64)

    inv = 1.0 / (ROPE_THETA ** (np.arange(0, D, 2, dtype=f32) / D))
    freqs = pos[:, None].astype(f32) * inv[None, :]          # [T, 64]
    emb = np.concatenate([freqs, freqs], axis=1)             # [T, 128]
    # QSCALE compensates the fp8 scaling of the Q projection inputs
    cos = (np.cos(emb) * QSCALE).astype(f32)
    sin = (np.sin(emb) * QSCALE).astype(f32)
    # sin arm pairs with the row-swapped KT: +sin (d<64), -sin (d>=64)
    sin_signed = sin.copy()
    sin_signed[:, D // 2:] *= -1.0

    gate = f32(np.asarray(adaption_gate).reshape(-1)[0])
    scale = f32(1.0 / np.sqrt(D))
    prompt = np.asarray(adaption_prompt, f32).reshape(L, HID)

    def tile_doublerow(A):
        # A [K, N] -> [N, K] tiled.
        # DoubleRow:       [128n+p, 256b+128i+c]    = A[256b+128i+p, 128n+c]
        # SwInterleave:    [128n+p, 256b+2(127-c)+i] = A[256b+128i+p, 128n+c]
        K, N = A.shape
        t = A.reshape(K // 256, 2, 128, N // 128, 128).transpose(3, 2, 0, 1, 4)
        if SWI:                       # (n, p, b, i, c) -> (n, p, b, 127-c, i)
            t = t[..., ::-1].transpose(0, 1, 2, 4, 3)
        return np.ascontiguousarray(t.reshape(N, K))

    def tile_dr_rhs(A):
        # A [HID, n] -> [128, KC*n], cols (k2, mc, i, m):
        # [p, k2*2n + mc*1024 + i*512 + m] = A[256k2+128i+p, 512mc+m]
        n = A.shape[1]
        return np.ascontiguousarray(
            A.reshape(KC // 2, 2, 128, n // 512, 512)
             .transpose(2, 0, 3, 1, 4).reshape(128, KC * n))

    WqT = tile_doublerow(np.asarray(Wq, f32).T * (scale * f32(S_Q)))
    WqT = to_fp8(WqT)

    # adapter K (host fold): K = prompt @ Wk.T, per head [KT | KT row-swapped]
    Kmat = prompt @ np.asarray(Wk, f32).T                    # [L, HID]
    KT = Kmat.reshape(L, H, D).transpose(2, 1, 0)            # [D, H, L]
    ktp = np.empty((128, H, 2, L), f32)
    ktp[:, :, 0, :] = KT * S_K
    ktp[:, :, 1, :] = np.roll(KT, -D // 2, axis=0) * S_K
    ktpT = to_fp8(ktp.reshape(128, H * 2 * L))

    # folded V@Wo (host): VWo[h] = (prompt @ Wv.T * gate)[:, h] @ Wo.T[h]
    V = (prompt @ np.asarray(Wv, f32).T) * gate              # [L, HID]
    V5 = V.reshape(L, H, D).transpose(1, 0, 2)               # [H, L, D]
    WoT5 = np.asarray(Wo, f32).T.reshape(H, D, HID)          # [H, D, HID]
    M2 = (V5 @ WoT5).reshape(HL, HID)                        # [(h,l), HID]
    vwoT = to_fp8(tile_doublerow(M2 * f32(S_VW)))

    u2 = np.zeros((128, 64), f32)
    u2[0:64, 31] = 1.0 / S_PR
    u2[64:128, 32] = 1.0 / S_PR
    z2 = np.zeros((32, 33 * 64), f32)
    for k in range(32):
        z2[k, 64 * k:64 * (k + 1)] = 1.0
    u2 = u2.astype(bf16)
    z2 = z2.astype(bf16)

    in_maps = []
    for c in range(ncores):
        lo = c * tc_tokens
        hi = lo + tc_tokens
        in_maps.append({
            "xT": tile_dr_rhs(to_fp8(x[lo:hi].T * f32(S_X))),
            "cosT": np.ascontiguousarray(cos[lo:hi].T).astype(bf16),
            "sinT": np.ascontiguousarray(sin_signed[lo:hi].T).astype(bf16),
            "wqT": WqT,
            "ktpT": ktpT,
            "vwoT": vwoT,
            "baseT": np.ascontiguousarray(base[lo:hi].T).astype(bf16),
            "u2T": u2,
            "z2T": z2,
        })
    return in_maps


def kernel(hidden_states, base_output, Wq, Wk, Wv, Wo, adaption_prompt,
           adaption_gate, position_ids):
    from concourse import bass_utils

    if "nc" not in _cache:
        _cache["nc"] = _build()
    nc = _cache["nc"]

    in_maps = _host_prep(hidden_states, base_output, Wq, Wk, Wv, Wo,
                         adaption_prompt, adaption_gate, position_ids)

    res = bass_utils.run_bass_kernel_spmd(nc, in_maps, core_ids=list(range(NCORES)))

    out = np.empty((T, HID), np.float32)
    for c in range(NCORES):
        out[c * TC:(c + 1) * TC] = res.results[c]["outT"].T
    return out.reshape(B, S, HID)
